# revision 1
# baseline (speedup 1.0000x reference)
"""KMeans dataset imputation on 8 Trainium2 NeuronCores.

Math: for each batch row b (masked squared distance to 512 centers):
    d[b,k] = sum_j m[b,j]*(x[b,j]-c[k,j])^2 = xx[b] - 2*xc[b,k] + cc[b,k]
argmin_k d is independent of xx[b], so we compute
    s[b,k] = 2*xc - cc = (m*x) @ w1 + m @ w2,   w1 = 2c^T, w2 = -(c^2)^T
and take argmax_k s. Output row = bank[argmax], bank = data_to_impute[per_cluster_index].

Precision: fp32 matmul on trn2 PE lowers to 4 bf16-rate passes (fp32_mode=LOW_HIGH).
We instead run 5 explicit bf16 passes with hi/lo operand splits:
    s = mxh@w1h + mxh@w1l + mxl@w1h + m@w2h + m@w2l
where m is {0,1} (exact in bf16) and mxh = m*bf16(x), mxl = m*bf16(x - hi(x))
are exact products. Dropped term mxl@w1l ~ 2^-18 relative: verified 0 argmin
flips vs fp64 on the real inputs (vs 7 flips when also dropping m@w2l).

Sharding: data-parallel over batch, 1024 rows/core; weights + 512-row bank
replicated. Host pre-transposes x, m to d-major so contraction lands on
SBUF partitions.
"""

from contextlib import ExitStack

import ml_dtypes
import numpy as np

import concourse.bass as bass
import concourse.tile as tile
from concourse import bacc, mybir
from concourse.bass_utils import run_bass_kernel_spmd

N_CORES = 8
B, D, K = 8192, 784, 512
BL = B // N_CORES          # 1024 batch rows per core
P = 128
ND = (D + P - 1) // P      # 7 contraction chunks (6x128 + 1x16)
NB = BL // P               # 8 batch tiles per core

f32 = mybir.dt.float32
bf16 = mybir.dt.bfloat16
np_bf16 = ml_dtypes.bfloat16

_last_results = None  # test harness reads exec_time_ns from here


def _build():
    nc = bacc.Bacc("TRN2", debug=False, num_devices=N_CORES)
    xh = nc.dram_tensor("xh", [D, BL], bf16, kind="ExternalInput").ap()
    xl = nc.dram_tensor("xl", [D, BL], bf16, kind="ExternalInput").ap()
    mb = nc.dram_tensor("mb", [D, BL], bf16, kind="ExternalInput").ap()
    w1h = nc.dram_tensor("w1h", [D, K], bf16, kind="ExternalInput").ap()
    w1l = nc.dram_tensor("w1l", [D, K], bf16, kind="ExternalInput").ap()
    w2h = nc.dram_tensor("w2h", [D, K], bf16, kind="ExternalInput").ap()
    w2l = nc.dram_tensor("w2l", [D, K], bf16, kind="ExternalInput").ap()
    bank = nc.dram_tensor("bank", [K, D], f32, kind="ExternalInput").ap()
    out = nc.dram_tensor("out", [BL, D], f32, kind="ExternalOutput").ap()

    with tile.TileContext(nc) as tc, ExitStack() as ctx:
        io = ctx.enter_context(tc.tile_pool(name="io", bufs=1))
        epi = ctx.enter_context(tc.tile_pool(name="epi", bufs=4))
        psp = ctx.enter_context(tc.tile_pool(name="psp", bufs=8, space="PSUM"))

        # Dep-free warm-up matmuls: keep the PE busy during the initial DMA
        # wait so HAM un-throttles (K=8/8) before the real matmuls start.
        warm = io.tile([P, K], bf16, tag="warm")
        nc.gpsimd.memset(warm[:], 0)
        wps = psp.tile([P, K], f32, tag="ps", name="wps")
        for _ in range(10):
            nc.tensor.matmul(wps[:], warm[:, :P], warm[:], start=True, stop=True)

        mxh_t, mxl_t, mb_t, w_t, djs = [], [], [], [], []
        for j in range(ND):
            dj = min(P, D - j * P)
            djs.append(dj)
            sl = slice(j * P, j * P + dj)
            # load order = first-use order: pass 1-2 need mxh(mb,xh), w1h, w1l
            mb_j = io.tile([P, BL], bf16, tag=f"mb{j}")
            nc.sync.dma_start(mb_j[:dj], mb[sl, :])
            xh_j = io.tile([P, BL], bf16, tag=f"xh{j}")
            nc.sync.dma_start(xh_j[:dj], xh[sl, :])
            ws = []
            for wi, wd in enumerate((w1h, w1l)):
                w_j = io.tile([P, K], bf16, tag=f"w{wi}_{j}", name=f"w{wi}_{j}")
                nc.sync.dma_start(w_j[:dj], wd[sl, :])
                ws.append(w_j)
            xl_j = io.tile([P, BL], bf16, tag=f"xl{j}")
            nc.sync.dma_start(xl_j[:dj], xl[sl, :])
            for wi, wd in ((2, w2h), (3, w2l)):
                w_j = io.tile([P, K], bf16, tag=f"w{wi}_{j}", name=f"w{wi}_{j}")
                nc.sync.dma_start(w_j[:dj], wd[sl, :])
                ws.append(w_j)
            mxh_j = io.tile([P, BL], bf16, tag=f"mxh{j}")
            nc.vector.tensor_mul(mxh_j[:dj], mb_j[:dj], xh_j[:dj])
            mxl_j = io.tile([P, BL], bf16, tag=f"mxl{j}")
            nc.vector.tensor_mul(mxl_j[:dj], mb_j[:dj], xl_j[:dj])
            mxh_t.append(mxh_j)
            mxl_t.append(mxl_j)
            mb_t.append(mb_j)
            w_t.append(ws)

        for b in range(NB):
            ps = psp.tile([P, K], f32, tag="ps", name=f"ps{b}")
            bsl = bass.ts(b, P)
            for j in range(ND):
                dj = djs[j]
                w1h_j, w1l_j, w2h_j, w2l_j = w_t[j]
                # same-lhsT passes adjacent: mxh (x2), mxl, mb (x2)
                nc.tensor.matmul(ps[:], mxh_t[j][:dj, bsl], w1h_j[:dj],
                                 start=(j == 0), stop=False)
                nc.tensor.matmul(ps[:], mxh_t[j][:dj, bsl], w1l_j[:dj],
                                 start=False, stop=False)
                nc.tensor.matmul(ps[:], mxl_t[j][:dj, bsl], w1h_j[:dj],
                                 start=False, stop=False)
                nc.tensor.matmul(ps[:], mb_t[j][:dj, bsl], w2h_j[:dj],
                                 start=False, stop=False)
                nc.tensor.matmul(ps[:], mb_t[j][:dj, bsl], w2l_j[:dj],
                                 start=False, stop=(j == ND - 1))

            sc = epi.tile([P, K], f32, tag="sc")
            nc.scalar.copy(sc[:], ps[:])
            mx8 = epi.tile([P, 8], f32, tag="mx8")
            nc.vector.max(mx8[:], sc[:])
            idx8 = epi.tile([P, 8], mybir.dt.uint32, tag="idx8")
            nc.vector.max_index(idx8[:], mx8[:], sc[:])
            g = epi.tile([P, D], f32, tag="g")
            nc.gpsimd.indirect_dma_start(
                out=g[:],
                out_offset=None,
                in_=bank[:],
                in_offset=bass.IndirectOffsetOnAxis(ap=idx8[:, :1], axis=0),
            )
            nc.sync.dma_start(out[b * P : (b + 1) * P, :], g[:])

    nc.compile()
    return nc


def _split(a):
    hi = a.astype(np_bf16)
    lo = (a - hi.astype(np.float32)).astype(np_bf16)
    return hi, lo


def kernel(data, mask, centers, data_to_impute, per_cluster_index):
    global _last_results
    x = np.asarray(data, dtype=np.float32).reshape(B, D).T    # [784, 8192]
    m = np.asarray(mask, dtype=np.float32).reshape(B, D).T
    c = np.asarray(centers, dtype=np.float32)
    w1h_h, w1l_h = _split(np.ascontiguousarray((2.0 * c).T))
    w2h_h, w2l_h = _split(np.ascontiguousarray((-(c * c)).T))
    pci = np.asarray(per_cluster_index).astype(np.int64)
    bank_h = np.ascontiguousarray(np.asarray(data_to_impute, dtype=np.float32)[pci])

    xh_h, xl_h = _split(x)
    mb_h = m.astype(np_bf16)

    in_maps = []
    for core in range(N_CORES):
        sl = slice(core * BL, (core + 1) * BL)
        in_maps.append(
            {
                "xh": np.ascontiguousarray(xh_h[:, sl]),
                "xl": np.ascontiguousarray(xl_h[:, sl]),
                "mb": np.ascontiguousarray(mb_h[:, sl]),
                "w1h": w1h_h,
                "w1l": w1l_h,
                "w2h": w2h_h,
                "w2l": w2l_h,
                "bank": bank_h,
            }
        )

    nc = _build()
    res = run_bass_kernel_spmd(nc, in_maps, core_ids=list(range(N_CORES)))
    _last_results = res
    out = np.concatenate([res.results[cc]["out"] for cc in range(N_CORES)], axis=0)
    return out.reshape(np.asarray(data).shape).astype(np.float32)

